# revision 23
# baseline (speedup 1.0000x reference)
"""Causal self-attention (B=4, T=2048, C=1024, H=16) on 8 TRN2 NeuronCores.

Sharding: 2-D data x tensor parallel. Core c handles batch b = c//2 and
head-group hg = c%2 (8 of 16 heads). Each core computes its local QKV
projection (c_attn columns for its heads), causal attention for its 8 heads,
and a partial c_proj (rows for its heads). The pair of cores sharing a batch
produce partial outputs that are summed on the host (the gather step).

All matmul inputs are bf16 (cast on host / on-chip), accumulation is fp32 in
PSUM. 1/sqrt(D) is folded into W_q on the host. Softmax skips the max
subtraction: with this input distribution |scores| <= ||q||*||k||/8, far from
fp32 exp overflow. Causal masking is a post-exp multiply with a 0/1 bf16
triangular mask on diagonal tiles.

Per-core kernel (matmuls contract over the SBUF partition dim):
  QKV:  qT/kT chunks [d_local, t] = W.T @ x.T; v tiles [t, d] with a ones
        column per head (PV then computes the softmax denominator for free).
  S^T:  [k, q] tiles = K @ Q^T; the two heads of a chunk pair use array rows
        0-63 / 64-127 and run concurrently (PE row tiling).
  PV:   O^T [d+1, q] accumulated over k tiles; row 64 = rowsum; normalize
        with reciprocal_approx_fast + partition_broadcast + multiply.
  proj: out[t, c_out] partial accumulated over the 512 local c_in rows.
"""

import os
import sys

import numpy as np

for _p in ("/opt/trn_rl_repo", "/root/.axon_site/_ro/trn_rl_repo"):
    if os.path.isdir(_p) and _p not in sys.path:
        sys.path.append(_p)

import ml_dtypes
import concourse.bacc as bacc
import concourse.mybir as mybir
from concourse.tile import TileContext
from concourse.bass_utils import run_bass_kernel_spmd

B, T, C, H, D = 4, 2048, 1024, 16, 64
HL = 8            # heads per core
CL = HL * D       # 512: local qkv width
NCT = C // 128    # 8 contraction tiles over C
NQC = T // 512    # 4 query chunks
NKT = T // 128    # 16 key tiles
NCORES = 8

FP32 = mybir.dt.float32
BF16 = mybir.dt.bfloat16

LAST_EXEC_NS = None
_CACHE = {}


def _act_reciprocal(nc, out_ap, in_ap):
    """Raw ACT Reciprocal (bass's activation() refuses it; ~1e-5 rel err is
    plenty for softmax denominators and it is 5x cheaper than DVE reciprocal)."""
    inst = mybir.InstActivation(
        name=nc.get_next_instruction_name(),
        func=mybir.ActivationFunctionType.Reciprocal,
        ins=[nc.scalar.lower_ap(in_ap),
             mybir.ImmediateValue(dtype=mybir.dt.float32, value=0.0),
             mybir.ImmediateValue(dtype=mybir.dt.float32, value=1.0),
             mybir.ImmediateValue(dtype=mybir.dt.float32, value=0.0)],
        outs=[nc.scalar.lower_ap(out_ap)])
    return nc.scalar.add_instruction(inst)


def _build():
    nc = bacc.Bacc("TRN2", target_bir_lowering=False, debug=False)
    xT = nc.dram_tensor("xT", [C, T], BF16, kind="ExternalInput")
    wq = nc.dram_tensor("wq", [C, CL], BF16, kind="ExternalInput")
    wk = nc.dram_tensor("wk", [C, CL], BF16, kind="ExternalInput")
    wv = nc.dram_tensor("wv", [C, CL], BF16, kind="ExternalInput")
    wp = nc.dram_tensor("wp", [CL, C], BF16, kind="ExternalInput")
    bqk = nc.dram_tensor("bqk", [128, 8], FP32, kind="ExternalInput")
    bv = nc.dram_tensor("bv", [1, CL], FP32, kind="ExternalInput")
    maskb = nc.dram_tensor("maskb", [128, 4, 2, 512], BF16, kind="ExternalInput")
    out = nc.dram_tensor("out", [T, C], FP32, kind="ExternalOutput")

    EXP = mybir.ActivationFunctionType.Exp

    with TileContext(nc) as tc:
        with (
            tc.tile_pool(name="persist", bufs=1) as pp,
            tc.tile_pool(name="xtp", bufs=1) as xt_pool,
            tc.tile_pool(name="wc", bufs=4) as wc_pool,
            tc.tile_pool(name="wvp", bufs=1) as wv_pool,
            tc.tile_pool(name="wpp", bufs=1) as wp_pool,
            tc.tile_pool(name="yt", bufs=1) as yt_pool,
            tc.tile_pool(name="pt", bufs=8) as pt_pool,
            tc.tile_pool(name="nrm", bufs=6) as nrm_pool,
            tc.tile_pool(name="stg", bufs=3) as stg_pool,
            tc.tile_pool(name="psS", bufs=2, space="PSUM") as psS,   # attention S^T pairs
            tc.tile_pool(name="psF", bufs=2, space="PSUM") as psF,   # qkv/v/proj groups
            tc.tile_pool(name="psO", bufs=2, space="PSUM") as psO,   # PV accumulators
        ):
            # persistent SBUF; qkT chunks split per 1024-wide half for finer deps
            qkTn = [[pp.tile([128, 1024], BF16, name=f"qkT{m}_{np_}")
                     for np_ in range(2)] for m in range(8)]
            vt = [pp.tile([128, HL, D + 1], BF16, name=f"v{i}") for i in range(NKT)]
            bqk_sb = pp.tile([128, 8], FP32, name="bqk_sb")
            bv1_sb = pp.tile([1, CL], FP32, name="bv1_sb")
            bvb_sb = pp.tile([128, CL], FP32, name="bvb_sb")
            mask_sb = pp.tile([128, 4, 2, 512], BF16, name="mask_sb")

            nc.gpsimd.dma_start(bqk_sb[:], bqk[:])
            nc.gpsimd.dma_start(bv1_sb[:], bv[:])
            nc.gpsimd.dma_start(mask_sb[:], maskb[:])
            nc.gpsimd.partition_broadcast(bvb_sb[:], bv1_sb[:])
            for i in range(NKT):
                nc.vector.memset(vt[i][:, :, D:D + 1], 1.0)

            def wt_load(wdram, m):
                wt = wc_pool.tile([128, NCT, 128], BF16, name="wt", tag="wt")
                nc.gpsimd.dma_start(
                    wt[:],
                    wdram[:, m * 128:(m + 1) * 128].rearrange("(a p) n -> p a n", p=128),
                )
                return wt

            wt_q0 = wt_load(wq, 0)
            wt_k0 = wt_load(wk, 0)

            xt = [xt_pool.tile([128, T], BF16, name=f"xt{ct}") for ct in range(NCT)]
            for ct in range(NCT):
                nc.sync.dma_start(xt[ct][:], xT[ct * 128:(ct + 1) * 128, :])
            wvt = wv_pool.tile([128, NCT, CL], BF16, name="wvt")
            nc.gpsimd.dma_start(wvt[:], wv.rearrange("(a p) n -> p a n", p=128))

            # absorber: PE observes xt0's DMA queue before the first real
            # matmul (limited sync-wait slots on the weight-load path)
            dummy = psO.tile([1, 64], FP32, name="po", tag="po")
            nc.tensor.matmul(dummy[:], xt[0][:, 0:1], xt[0][:, 0:64])

            def qk_half(m, col, wt, np_):
                """One 1024-wide half of a q/k chunk: two 8-matmul groups."""
                for half in range(2):
                    n = 2 * np_ + half
                    ps = psF.tile([128, 512], FP32, name="fg", tag="fg")
                    for ct in range(NCT):
                        nc.tensor.matmul(
                            ps[:],
                            wt[:, ct, :],
                            xt[ct][:, n * 512:(n + 1) * 512],
                            start=(ct == 0), stop=(ct == NCT - 1),
                        )
                    nc.vector.tensor_scalar_add(
                        qkTn[col][np_][:, half * 512:(half + 1) * 512], ps[:],
                        bqk_sb[:, col:col + 1])

            def qk_half_gen(m, col, wt, np_):
                """Generator version: yields every 2 matmuls (feeder unit)."""
                for half in range(2):
                    n = 2 * np_ + half
                    ps = psF.tile([128, 512], FP32, name="fg", tag="fg")
                    for cp in range(NCT // 2):
                        for ct in (2 * cp, 2 * cp + 1):
                            nc.tensor.matmul(
                                ps[:],
                                wt[:, ct, :],
                                xt[ct][:, n * 512:(n + 1) * 512],
                                start=(ct == 0), stop=(ct == NCT - 1),
                            )
                        yield
                    nc.vector.tensor_scalar_add(
                        qkTn[col][np_][:, half * 512:(half + 1) * 512], ps[:],
                        bqk_sb[:, col:col + 1])
                    yield

            def v_group(ip):
                for half in range(2):
                    i = 2 * ip + half
                    ps = psF.tile([128, 512], FP32, name="fg", tag="fg")
                    for ct in range(NCT):
                        nc.tensor.matmul(
                            ps[:],
                            xt[ct][:, i * 128:(i + 1) * 128],
                            wvt[:, ct, :],
                            start=(ct == 0), stop=(ct == NCT - 1),
                        )
                    nc.vector.tensor_add(
                        vt[i][:, :, 0:D],
                        ps[:].rearrange("p (h d) -> p h d", h=HL),
                        bvb_sb[:].rearrange("p (h d) -> p h d", h=HL),
                    )

            feeder = []

            def pump(n):
                while n > 0 and feeder:
                    try:
                        next(feeder[0])
                        n -= 1
                    except StopIteration:
                        feeder.pop(0)

            yT = [yt_pool.tile([128, 4, 512], BF16, name=f"yT{j}") for j in range(NQC)]

            def attention(m, fillers=None, order=None):
                kcol = 4 + m
                for idx, j in enumerate(order or range(NQC)):
                    for fn in (fillers or {}).get(idx, []):
                        fn()
                    po = [psO.tile([65, 512], FP32, name="po", tag="po")
                          for _ in range(2)]
                    npair = 2 * (j + 1)
                    for kp in range(npair):
                        kts = (2 * kp, 2 * kp + 1)
                        starts = [min(max(0, 128 * (kts[half] - 4 * j)), 512)
                                  for half in range(2)]
                        # per k-tile (half): one [128,1024] tile holding both
                        # heads side by side (separate banks) so the two K=64
                        # matmuls are adjacent and row-pair on the PE array
                        ps = [psS.tile([128, 1024], FP32, name="st", tag="st")
                              for _ in range(2)]
                        for half in range(2):
                            s = starts[half]
                            kt = kts[half]
                            for hh in range(2):
                                pb = hh * 64
                                nc.tensor.matmul(
                                    ps[half][:, hh * 512 + s:(hh + 1) * 512],
                                    qkTn[kcol][kt // 8][pb:pb + 64,
                                        (kt % 8) * 128:(kt % 8 + 1) * 128],
                                    qkTn[m][j // 2][pb:pb + 64,
                                        (j % 2) * 512 + s:(j % 2 + 1) * 512],
                                    start=True, stop=True,
                                    tile_position=(pb, 0),
                                )
                        pt = [pt_pool.tile([128, 1024], BF16, name="pt", tag="pt")
                              for _ in range(2)]
                        for half in range(2):
                            s = starts[half]
                            if s == 0:
                                nc.scalar.activation(pt[half][:], ps[half][:], EXP)
                            else:
                                nc.scalar.activation(
                                    pt[half][:].rearrange(
                                        "p (h n) -> p h n", h=2)[:, :, s:512],
                                    ps[half][:].rearrange(
                                        "p (h n) -> p h n", h=2)[:, :, s:512],
                                    EXP)
                            if kts[half] >= 4 * j:
                                jj = kts[half] - 4 * j
                                pv3 = pt[half][:].rearrange(
                                    "p (h n) -> p h n", h=2)[:, :, s:512]
                                nc.vector.tensor_mul(
                                    pv3, pv3, mask_sb[:, jj, :, s:512])
                        for hh in range(2):
                            h = 2 * m + hh
                            for half in range(2):
                                s = starts[half]
                                nc.tensor.matmul(
                                    po[hh][:, s:512],
                                    vt[kts[half]][:, h, :],
                                    pt[half][:, hh * 512 + s:(hh + 1) * 512],
                                    start=(kp == 0 and half == 0),
                                    stop=(kp == npair - 1 and half == 1),
                                )
                        pump(3 if idx < 2 else 2)
                    for hh in range(2):
                        pb = hh * 64
                        # copy out of PSUM right away so the po slot frees
                        oc = nrm_pool.tile([64, 512], FP32, name="oc", tag="oc")
                        nc.vector.tensor_copy(oc[:], po[hh][0:64, :])
                        rs = nrm_pool.tile([1, 512], FP32, name="rs", tag="rs")
                        nc.vector.tensor_copy(rs[:], po[hh][64:65, :])
                        recip = nrm_pool.tile([1, 512], FP32, name="recip", tag="recip")
                        nc.vector.reciprocal_approx_fast(recip[:], rs[:])
                        rb = nrm_pool.tile([64, 512], FP32, name="rb", tag="rb")
                        nc.gpsimd.partition_broadcast(rb[:], recip[:])
                        nc.vector.tensor_mul(
                            yT[j][pb:pb + 64, m, :], oc[:], rb[:]
                        )

            # ---------------- proj ----------------
            wpt = wp_pool.tile([128, 4, C], BF16, name="wpt")

            def proj_block(j):
                for nn in range(2):
                    for ts in range(4):
                        ps = psF.tile([128, 512], FP32, name="fg", tag="fg")
                        for cl in range(4):
                            nc.tensor.matmul(
                                ps[:],
                                yT[j][:, cl, ts * 128:(ts + 1) * 128],
                                wpt[:, cl, nn * 512:(nn + 1) * 512],
                                start=(cl == 0), stop=(cl == 3),
                            )
                        stage = stg_pool.tile([128, 512], FP32, name="stage", tag="stage")
                        nc.vector.tensor_copy(stage[:], ps[:])
                        nc.sync.dma_start(
                            out[j * 512 + ts * 128:j * 512 + (ts + 1) * 128,
                                nn * 512:(nn + 1) * 512],
                            stage[:],
                        )

            def qk_half_ctmajor(m, col, wt, np_):
                """ct-outer over both 512-halves: PE can start as soon as
                xt[0] lands instead of waiting for the whole x stream."""
                pss = [psF.tile([128, 512], FP32, name="fg", tag="fg")
                       for _ in range(2)]
                for ct in range(NCT):
                    for half in range(2):
                        n = 2 * np_ + half
                        nc.tensor.matmul(
                            pss[half][:],
                            wt[:, ct, :],
                            xt[ct][:, n * 512:(n + 1) * 512],
                            start=(ct == 0), stop=(ct == NCT - 1),
                        )
                for half in range(2):
                    nc.vector.tensor_scalar_add(
                        qkTn[col][np_][:, half * 512:(half + 1) * 512],
                        pss[half][:], bqk_sb[:, col:col + 1])

            # ---------- schedule ----------
            qk_half_ctmajor(0, 0, wt_q0, 0)      # q0 lower half
            qk_half_ctmajor(0, 4, wt_k0, 0)      # k0 lower half
            v_group(0)
            v_group(1)
            wt_q1 = wt_load(wq, 1)
            wt_k1 = wt_load(wk, 1)
            feeder.append(qk_half_gen(1, 1, wt_q1, 0))
            feeder.append(qk_half_gen(1, 5, wt_k1, 0))
            feeder.append(qk_half_gen(1, 1, wt_q1, 1))
            feeder.append(qk_half_gen(1, 5, wt_k1, 1))
            attention(0, {
                1: [lambda: v_group(2), lambda: v_group(3),
                    lambda: qk_half(0, 0, wt_q0, 1)],
                2: [lambda: v_group(4), lambda: v_group(5),
                    lambda: qk_half(0, 4, wt_k0, 1)],
                3: [lambda: v_group(6), lambda: v_group(7)],
            })
            wt_q2 = wt_load(wq, 2)
            wt_k2 = wt_load(wk, 2)
            feeder.append(qk_half_gen(2, 2, wt_q2, 0))
            feeder.append(qk_half_gen(2, 6, wt_k2, 0))
            feeder.append(qk_half_gen(2, 2, wt_q2, 1))
            feeder.append(qk_half_gen(2, 6, wt_k2, 1))
            attention(1)
            wt_q3 = wt_load(wq, 3)
            wt_k3 = wt_load(wk, 3)
            nc.gpsimd.dma_start(wpt[:], wp.rearrange("(a p) n -> p a n", p=128))
            feeder.append(qk_half_gen(3, 3, wt_q3, 0))
            feeder.append(qk_half_gen(3, 7, wt_k3, 0))
            feeder.append(qk_half_gen(3, 3, wt_q3, 1))
            feeder.append(qk_half_gen(3, 7, wt_k3, 1))
            attention(2)
            attention(3, {
                1: [lambda: proj_block(3)],
                2: [lambda: proj_block(0)],
                3: [lambda: proj_block(1)],
            }, order=[3, 0, 1, 2])
            proj_block(2)
    nc.compile()
    return nc


def _host_inputs(x, W_attn, b_attn, W_proj):
    """Build the 8 per-core input maps (bf16 casts happen here)."""
    x = np.asarray(x, dtype=np.float32)
    W_attn = np.asarray(W_attn, dtype=np.float32)
    b_attn = np.asarray(b_attn, dtype=np.float32)
    W_proj = np.asarray(W_proj, dtype=np.float32)

    scale = np.float32(1.0 / np.sqrt(D))
    # causal mask for diagonal tiles: [p, jj, f] = 1 where f >= p + 128*jj
    p = np.arange(128)[:, None, None]
    jj = np.arange(4)[None, :, None]
    f = np.arange(512)[None, None, :]
    maskb = (f >= p + 128 * jj).astype(ml_dtypes.bfloat16)
    maskb = np.ascontiguousarray(np.repeat(maskb[:, :, None, :], 2, axis=2))

    bf = ml_dtypes.bfloat16
    in_maps = []
    xT_b = [np.ascontiguousarray(x[b].T.astype(bf)) for b in range(B)]
    for c in range(NCORES):
        b, hg = c // 2, c % 2
        lo = hg * CL
        wq_np = np.ascontiguousarray((W_attn[:, lo:lo + CL] * scale).astype(bf))
        wk_np = np.ascontiguousarray(W_attn[:, C + lo:C + lo + CL].astype(bf))
        wv_np = np.ascontiguousarray(W_attn[:, 2 * C + lo:2 * C + lo + CL].astype(bf))
        wp_np = np.ascontiguousarray(W_proj[lo:lo + CL, :].astype(bf))
        bq = (b_attn[lo:lo + CL] * scale).reshape(4, 128).T          # [128, 4]
        bk = b_attn[C + lo:C + lo + CL].reshape(4, 128).T
        bqk_np = np.ascontiguousarray(np.concatenate([bq, bk], axis=1), dtype=np.float32)
        bv_np = np.ascontiguousarray(b_attn[2 * C + lo:2 * C + lo + CL].reshape(1, CL))
        in_maps.append({
            "xT": xT_b[b],
            "wq": wq_np, "wk": wk_np, "wv": wv_np, "wp": wp_np,
            "bqk": bqk_np, "bv": bv_np, "maskb": maskb,
        })
    return in_maps


def kernel(x, W_attn, b_attn, W_proj, b_proj):
    global LAST_EXEC_NS
    if "nc" not in _CACHE:
        _CACHE["nc"] = _build()
    nc = _CACHE["nc"]
    in_maps = _host_inputs(x, W_attn, b_attn, W_proj)
    trace = os.environ.get("KERNEL_TRACE", "0") == "1"
    kwargs = {}
    if trace:
        kwargs["trace"] = True
        td = os.environ.get("KERNEL_TRACE_DIR")
        if td:
            kwargs["tmpdir"] = td
    res = run_bass_kernel_spmd(nc, in_maps, list(range(NCORES)), **kwargs)
    LAST_EXEC_NS = res.exec_time_ns
    b_proj = np.asarray(b_proj, dtype=np.float32)
    outs = []
    for b in range(B):
        outs.append(res.results[2 * b]["out"] + res.results[2 * b + 1]["out"] + b_proj)
    return np.stack(outs, axis=0).astype(np.float32)


# revision 24
# speedup vs baseline: 1.0165x; 1.0165x over previous
"""Causal self-attention (B=4, T=2048, C=1024, H=16) on 8 TRN2 NeuronCores.

Sharding: 2-D data x tensor parallel. Core c handles batch b = c//2 and
head-group hg = c%2 (8 of 16 heads). Each core computes its local QKV
projection (c_attn columns for its heads), causal attention for its 8 heads,
and a partial c_proj (rows for its heads). The pair of cores sharing a batch
produce partial outputs that are summed on the host (the gather step).

All matmul inputs are bf16 (cast on host / on-chip), accumulation is fp32 in
PSUM. 1/sqrt(D) is folded into W_q on the host. Softmax skips the max
subtraction: with this input distribution |scores| <= ||q||*||k||/8, far from
fp32 exp overflow. Causal masking is a post-exp multiply with a 0/1 bf16
triangular mask on diagonal tiles.

Per-core kernel (matmuls contract over the SBUF partition dim):
  QKV:  qT/kT chunks [d_local, t] = W.T @ x.T; v tiles [t, d] with a ones
        column per head (PV then computes the softmax denominator for free).
  S^T:  [k, q] tiles = K @ Q^T; the two heads of a chunk pair use array rows
        0-63 / 64-127 and run concurrently (PE row tiling).
  PV:   O^T [d+1, q] accumulated over k tiles; row 64 = rowsum; normalize
        with reciprocal_approx_fast + partition_broadcast + multiply.
  proj: out[t, c_out] partial accumulated over the 512 local c_in rows.
"""

import os
import sys

import numpy as np

for _p in ("/opt/trn_rl_repo", "/root/.axon_site/_ro/trn_rl_repo"):
    if os.path.isdir(_p) and _p not in sys.path:
        sys.path.append(_p)

import ml_dtypes
import concourse.bacc as bacc
import concourse.mybir as mybir
from concourse.tile import TileContext
from concourse.bass_utils import run_bass_kernel_spmd

B, T, C, H, D = 4, 2048, 1024, 16, 64
HL = 8            # heads per core
CL = HL * D       # 512: local qkv width
NCT = C // 128    # 8 contraction tiles over C
NQC = T // 512    # 4 query chunks
NKT = T // 128    # 16 key tiles
NCORES = 8

FP32 = mybir.dt.float32
BF16 = mybir.dt.bfloat16

LAST_EXEC_NS = None
_CACHE = {}


def _act_reciprocal(nc, out_ap, in_ap):
    """Raw ACT Reciprocal (bass's activation() refuses it; ~1e-5 rel err is
    plenty for softmax denominators and it is 5x cheaper than DVE reciprocal)."""
    inst = mybir.InstActivation(
        name=nc.get_next_instruction_name(),
        func=mybir.ActivationFunctionType.Reciprocal,
        ins=[nc.scalar.lower_ap(in_ap),
             mybir.ImmediateValue(dtype=mybir.dt.float32, value=0.0),
             mybir.ImmediateValue(dtype=mybir.dt.float32, value=1.0),
             mybir.ImmediateValue(dtype=mybir.dt.float32, value=0.0)],
        outs=[nc.scalar.lower_ap(out_ap)])
    return nc.scalar.add_instruction(inst)


def _build():
    nc = bacc.Bacc("TRN2", target_bir_lowering=False, debug=False)
    xT = nc.dram_tensor("xT", [C, T], BF16, kind="ExternalInput")
    wq = nc.dram_tensor("wq", [C, CL], BF16, kind="ExternalInput")
    wk = nc.dram_tensor("wk", [C, CL], BF16, kind="ExternalInput")
    wv = nc.dram_tensor("wv", [C, CL], BF16, kind="ExternalInput")
    wp = nc.dram_tensor("wp", [CL, C], BF16, kind="ExternalInput")
    bqk = nc.dram_tensor("bqk", [128, 8], FP32, kind="ExternalInput")
    bv = nc.dram_tensor("bv", [1, CL], FP32, kind="ExternalInput")
    maskb = nc.dram_tensor("maskb", [128, 4, 2, 512], BF16, kind="ExternalInput")
    out = nc.dram_tensor("out", [T, C], FP32, kind="ExternalOutput")

    EXP = mybir.ActivationFunctionType.Exp

    with TileContext(nc) as tc:
        with (
            tc.tile_pool(name="persist", bufs=1) as pp,
            tc.tile_pool(name="xtp", bufs=1) as xt_pool,
            tc.tile_pool(name="wc", bufs=4) as wc_pool,
            tc.tile_pool(name="wvp", bufs=1) as wv_pool,
            tc.tile_pool(name="wpp", bufs=1) as wp_pool,
            tc.tile_pool(name="yt", bufs=1) as yt_pool,
            tc.tile_pool(name="pt", bufs=8) as pt_pool,
            tc.tile_pool(name="nrm", bufs=6) as nrm_pool,
            tc.tile_pool(name="stg", bufs=3) as stg_pool,
            tc.tile_pool(name="psS", bufs=2, space="PSUM") as psS,   # attention S^T pairs
            tc.tile_pool(name="psF", bufs=2, space="PSUM") as psF,   # qkv/v/proj groups
            tc.tile_pool(name="psO", bufs=2, space="PSUM") as psO,   # PV accumulators
        ):
            # persistent SBUF; qkT chunks split per 1024-wide half for finer deps
            qkTn = [[pp.tile([128, 1024], BF16, name=f"qkT{m}_{np_}")
                     for np_ in range(2)] for m in range(8)]
            vt = [pp.tile([128, HL, D + 1], BF16, name=f"v{i}") for i in range(NKT)]
            bqk_sb = pp.tile([128, 8], FP32, name="bqk_sb")
            bv1_sb = pp.tile([1, CL], FP32, name="bv1_sb")
            bvb_sb = pp.tile([128, CL], FP32, name="bvb_sb")
            mask_sb = pp.tile([128, 4, 2, 512], BF16, name="mask_sb")

            nc.gpsimd.dma_start(bqk_sb[:], bqk[:])
            nc.gpsimd.dma_start(bv1_sb[:], bv[:])
            nc.gpsimd.dma_start(mask_sb[:], maskb[:])
            nc.gpsimd.partition_broadcast(bvb_sb[:], bv1_sb[:])
            for i in range(NKT):
                nc.vector.memset(vt[i][:, :, D:D + 1], 1.0)

            def wt_load(wdram, m):
                wt = wc_pool.tile([128, NCT, 128], BF16, name="wt", tag="wt")
                nc.gpsimd.dma_start(
                    wt[:],
                    wdram[:, m * 128:(m + 1) * 128].rearrange("(a p) n -> p a n", p=128),
                )
                return wt

            wt_q0 = wt_load(wq, 0)
            wt_k0 = wt_load(wk, 0)

            xt = [xt_pool.tile([128, T], BF16, name=f"xt{ct}") for ct in range(NCT)]
            for ct in range(NCT):
                nc.sync.dma_start(xt[ct][:], xT[ct * 128:(ct + 1) * 128, :])
            wvt = wv_pool.tile([128, NCT, CL], BF16, name="wvt")
            nc.gpsimd.dma_start(wvt[:], wv.rearrange("(a p) n -> p a n", p=128))

            # absorber: PE observes xt0's DMA queue before the first real
            # matmul (limited sync-wait slots on the weight-load path)
            dummy = psO.tile([1, 64], FP32, name="po", tag="po")
            nc.tensor.matmul(dummy[:], xt[0][:, 0:1], xt[0][:, 0:64])

            def qk_half(m, col, wt, np_):
                """One 1024-wide half of a q/k chunk: two 8-matmul groups."""
                for half in range(2):
                    n = 2 * np_ + half
                    ps = psF.tile([128, 512], FP32, name="fg", tag="fg")
                    for ct in range(NCT):
                        nc.tensor.matmul(
                            ps[:],
                            wt[:, ct, :],
                            xt[ct][:, n * 512:(n + 1) * 512],
                            start=(ct == 0), stop=(ct == NCT - 1),
                        )
                    nc.vector.tensor_scalar_add(
                        qkTn[col][np_][:, half * 512:(half + 1) * 512], ps[:],
                        bqk_sb[:, col:col + 1])

            def qk_half_gen(m, col, wt, np_):
                """Generator version: yields every 2 matmuls (feeder unit)."""
                for half in range(2):
                    n = 2 * np_ + half
                    ps = psF.tile([128, 512], FP32, name="fg", tag="fg")
                    for cp in range(NCT // 2):
                        for ct in (2 * cp, 2 * cp + 1):
                            nc.tensor.matmul(
                                ps[:],
                                wt[:, ct, :],
                                xt[ct][:, n * 512:(n + 1) * 512],
                                start=(ct == 0), stop=(ct == NCT - 1),
                            )
                        yield
                    nc.vector.tensor_scalar_add(
                        qkTn[col][np_][:, half * 512:(half + 1) * 512], ps[:],
                        bqk_sb[:, col:col + 1])
                    yield

            def v_group(ip):
                for half in range(2):
                    i = 2 * ip + half
                    ps = psF.tile([128, 512], FP32, name="fg", tag="fg")
                    for ct in range(NCT):
                        nc.tensor.matmul(
                            ps[:],
                            xt[ct][:, i * 128:(i + 1) * 128],
                            wvt[:, ct, :],
                            start=(ct == 0), stop=(ct == NCT - 1),
                        )
                    nc.vector.tensor_add(
                        vt[i][:, :, 0:D],
                        ps[:].rearrange("p (h d) -> p h d", h=HL),
                        bvb_sb[:].rearrange("p (h d) -> p h d", h=HL),
                    )

            feeder = []

            def pump(n):
                while n > 0 and feeder:
                    try:
                        next(feeder[0])
                        n -= 1
                    except StopIteration:
                        feeder.pop(0)

            yT = [yt_pool.tile([128, 4, 512], BF16, name=f"yT{j}") for j in range(NQC)]

            def attention(m, fillers=None, order=None):
                kcol = 4 + m
                for idx, j in enumerate(order or range(NQC)):
                    for fn in (fillers or {}).get(idx, []):
                        fn()
                    po = [psO.tile([65, 512], FP32, name="po", tag="po")
                          for _ in range(2)]
                    npair = 2 * (j + 1)
                    for kp in range(npair):
                        kts = (2 * kp, 2 * kp + 1)
                        starts = [min(max(0, 128 * (kts[half] - 4 * j)), 512)
                                  for half in range(2)]
                        # per k-tile (half): one [128,1024] tile holding both
                        # heads side by side (separate banks) so the two K=64
                        # matmuls are adjacent and row-pair on the PE array
                        ps = [psS.tile([128, 1024], FP32, name="st", tag="st")
                              for _ in range(2)]
                        for half in range(2):
                            s = starts[half]
                            kt = kts[half]
                            for hh in range(2):
                                pb = hh * 64
                                nc.tensor.matmul(
                                    ps[half][:, hh * 512 + s:(hh + 1) * 512],
                                    qkTn[kcol][kt // 8][pb:pb + 64,
                                        (kt % 8) * 128:(kt % 8 + 1) * 128],
                                    qkTn[m][j // 2][pb:pb + 64,
                                        (j % 2) * 512 + s:(j % 2 + 1) * 512],
                                    start=True, stop=True,
                                    tile_position=(pb, 0),
                                )
                        pt = [pt_pool.tile([128, 1024], BF16, name="pt", tag="pt")
                              for _ in range(2)]
                        for half in range(2):
                            s = starts[half]
                            if s == 0:
                                nc.scalar.activation(pt[half][:], ps[half][:], EXP)
                            else:
                                nc.scalar.activation(
                                    pt[half][:].rearrange(
                                        "p (h n) -> p h n", h=2)[:, :, s:512],
                                    ps[half][:].rearrange(
                                        "p (h n) -> p h n", h=2)[:, :, s:512],
                                    EXP)
                            if kts[half] >= 4 * j:
                                jj = kts[half] - 4 * j
                                pv3 = pt[half][:].rearrange(
                                    "p (h n) -> p h n", h=2)[:, :, s:512]
                                nc.vector.tensor_mul(
                                    pv3, pv3, mask_sb[:, jj, :, s:512])
                        for hh in range(2):
                            h = 2 * m + hh
                            for half in range(2):
                                s = starts[half]
                                nc.tensor.matmul(
                                    po[hh][:, s:512],
                                    vt[kts[half]][:, h, :],
                                    pt[half][:, hh * 512 + s:(hh + 1) * 512],
                                    start=(kp == 0 and half == 0),
                                    stop=(kp == npair - 1 and half == 1),
                                )
                        pump(3 if idx < 2 else 2)
                    for hh in range(2):
                        pb = hh * 64
                        # copy out of PSUM right away so the po slot frees
                        oc = nrm_pool.tile([64, 512], FP32, name="oc", tag="oc")
                        nc.vector.tensor_copy(oc[:], po[hh][0:64, :])
                        rs = nrm_pool.tile([1, 512], FP32, name="rs", tag="rs")
                        nc.vector.tensor_copy(rs[:], po[hh][64:65, :])
                        recip = nrm_pool.tile([1, 512], FP32, name="recip", tag="recip")
                        nc.vector.reciprocal_approx_fast(recip[:], rs[:])
                        rb = nrm_pool.tile([64, 512], FP32, name="rb", tag="rb")
                        nc.gpsimd.partition_broadcast(rb[:], recip[:])
                        nc.vector.tensor_mul(
                            yT[j][pb:pb + 64, m, :], oc[:], rb[:]
                        )

            # ---------------- proj ----------------
            wpt = wp_pool.tile([128, 4, C], BF16, name="wpt")

            def proj_block(j):
                for nn in range(2):
                    for ts in range(4):
                        ps = psF.tile([128, 512], FP32, name="fg", tag="fg")
                        for cl in range(4):
                            nc.tensor.matmul(
                                ps[:],
                                yT[j][:, cl, ts * 128:(ts + 1) * 128],
                                wpt[:, cl, nn * 512:(nn + 1) * 512],
                                start=(cl == 0), stop=(cl == 3),
                            )
                        stage = stg_pool.tile([128, 512], FP32, name="stage", tag="stage")
                        nc.vector.tensor_copy(stage[:], ps[:])
                        nc.sync.dma_start(
                            out[j * 512 + ts * 128:j * 512 + (ts + 1) * 128,
                                nn * 512:(nn + 1) * 512],
                            stage[:],
                        )

            def qk_half_ctmajor(m, col, wt, np_):
                """ct-outer over both 512-halves: PE can start as soon as
                xt[0] lands instead of waiting for the whole x stream."""
                pss = [psF.tile([128, 512], FP32, name="fg", tag="fg")
                       for _ in range(2)]
                for ct in range(NCT):
                    for half in range(2):
                        n = 2 * np_ + half
                        nc.tensor.matmul(
                            pss[half][:],
                            wt[:, ct, :],
                            xt[ct][:, n * 512:(n + 1) * 512],
                            start=(ct == 0), stop=(ct == NCT - 1),
                        )
                for half in range(2):
                    nc.vector.tensor_scalar_add(
                        qkTn[col][np_][:, half * 512:(half + 1) * 512],
                        pss[half][:], bqk_sb[:, col:col + 1])

            # ---------- schedule ----------
            qk_half_ctmajor(0, 0, wt_q0, 0)      # q0 lower half
            qk_half_ctmajor(0, 4, wt_k0, 0)      # k0 lower half
            v_group(0)
            v_group(1)
            wt_q1 = wt_load(wq, 1)
            wt_k1 = wt_load(wk, 1)
            feeder.append(qk_half_gen(1, 1, wt_q1, 0))
            feeder.append(qk_half_gen(1, 5, wt_k1, 0))
            feeder.append(qk_half_gen(1, 1, wt_q1, 1))
            feeder.append(qk_half_gen(1, 5, wt_k1, 1))
            attention(0, {
                1: [lambda: v_group(2), lambda: v_group(3),
                    lambda: qk_half(0, 0, wt_q0, 1)],
                2: [lambda: v_group(4), lambda: v_group(5),
                    lambda: qk_half(0, 4, wt_k0, 1)],
                3: [lambda: v_group(6), lambda: v_group(7)],
            })
            wt_q2 = wt_load(wq, 2)
            wt_k2 = wt_load(wk, 2)
            feeder.append(qk_half_gen(2, 2, wt_q2, 0))
            feeder.append(qk_half_gen(2, 6, wt_k2, 0))
            feeder.append(qk_half_gen(2, 2, wt_q2, 1))
            feeder.append(qk_half_gen(2, 6, wt_k2, 1))
            attention(1)
            wt_q3 = wt_load(wq, 3)
            wt_k3 = wt_load(wk, 3)
            nc.gpsimd.dma_start(wpt[:], wp.rearrange("(a p) n -> p a n", p=128))
            feeder.append(qk_half_gen(3, 3, wt_q3, 0))
            feeder.append(qk_half_gen(3, 7, wt_k3, 0))
            feeder.append(qk_half_gen(3, 3, wt_q3, 1))
            feeder.append(qk_half_gen(3, 7, wt_k3, 1))
            attention(2)
            attention(3, {
                1: [lambda: proj_block(0)],
                2: [lambda: proj_block(1)],
                3: [lambda: proj_block(2)],
            })
            proj_block(3)
    nc.compile()
    return nc


def _host_inputs(x, W_attn, b_attn, W_proj):
    """Build the 8 per-core input maps (bf16 casts happen here)."""
    x = np.asarray(x, dtype=np.float32)
    W_attn = np.asarray(W_attn, dtype=np.float32)
    b_attn = np.asarray(b_attn, dtype=np.float32)
    W_proj = np.asarray(W_proj, dtype=np.float32)

    scale = np.float32(1.0 / np.sqrt(D))
    # causal mask for diagonal tiles: [p, jj, f] = 1 where f >= p + 128*jj
    p = np.arange(128)[:, None, None]
    jj = np.arange(4)[None, :, None]
    f = np.arange(512)[None, None, :]
    maskb = (f >= p + 128 * jj).astype(ml_dtypes.bfloat16)
    maskb = np.ascontiguousarray(np.repeat(maskb[:, :, None, :], 2, axis=2))

    bf = ml_dtypes.bfloat16
    in_maps = []
    xT_b = [np.ascontiguousarray(x[b].T.astype(bf)) for b in range(B)]
    for c in range(NCORES):
        b, hg = c // 2, c % 2
        lo = hg * CL
        wq_np = np.ascontiguousarray((W_attn[:, lo:lo + CL] * scale).astype(bf))
        wk_np = np.ascontiguousarray(W_attn[:, C + lo:C + lo + CL].astype(bf))
        wv_np = np.ascontiguousarray(W_attn[:, 2 * C + lo:2 * C + lo + CL].astype(bf))
        wp_np = np.ascontiguousarray(W_proj[lo:lo + CL, :].astype(bf))
        bq = (b_attn[lo:lo + CL] * scale).reshape(4, 128).T          # [128, 4]
        bk = b_attn[C + lo:C + lo + CL].reshape(4, 128).T
        bqk_np = np.ascontiguousarray(np.concatenate([bq, bk], axis=1), dtype=np.float32)
        bv_np = np.ascontiguousarray(b_attn[2 * C + lo:2 * C + lo + CL].reshape(1, CL))
        in_maps.append({
            "xT": xT_b[b],
            "wq": wq_np, "wk": wk_np, "wv": wv_np, "wp": wp_np,
            "bqk": bqk_np, "bv": bv_np, "maskb": maskb,
        })
    return in_maps


def kernel(x, W_attn, b_attn, W_proj, b_proj):
    global LAST_EXEC_NS
    if "nc" not in _CACHE:
        _CACHE["nc"] = _build()
    nc = _CACHE["nc"]
    in_maps = _host_inputs(x, W_attn, b_attn, W_proj)
    trace = os.environ.get("KERNEL_TRACE", "0") == "1"
    kwargs = {}
    if trace:
        kwargs["trace"] = True
        td = os.environ.get("KERNEL_TRACE_DIR")
        if td:
            kwargs["tmpdir"] = td
    res = run_bass_kernel_spmd(nc, in_maps, list(range(NCORES)), **kwargs)
    LAST_EXEC_NS = res.exec_time_ns
    b_proj = np.asarray(b_proj, dtype=np.float32)
    outs = []
    for b in range(B):
        outs.append(res.results[2 * b]["out"] + res.results[2 * b + 1]["out"] + b_proj)
    return np.stack(outs, axis=0).astype(np.float32)


# revision 25
# speedup vs baseline: 1.0283x; 1.0116x over previous
"""Causal self-attention (B=4, T=2048, C=1024, H=16) on 8 TRN2 NeuronCores.

Sharding: 2-D data x tensor parallel. Core c handles batch b = c//2 and
head-group hg = c%2 (8 of 16 heads). Each core computes its local QKV
projection (c_attn columns for its heads), causal attention for its 8 heads,
and a partial c_proj (rows for its heads). The pair of cores sharing a batch
produce partial outputs that are summed on the host (the gather step).

All matmul inputs are bf16 (cast on host / on-chip), accumulation is fp32 in
PSUM. 1/sqrt(D) is folded into W_q on the host. Softmax skips the max
subtraction: with this input distribution |scores| <= ||q||*||k||/8, far from
fp32 exp overflow. Causal masking is a post-exp multiply with a 0/1 bf16
triangular mask on diagonal tiles.

Per-core kernel (matmuls contract over the SBUF partition dim):
  QKV:  qT/kT chunks [d_local, t] = W.T @ x.T; v tiles [t, d] with a ones
        column per head (PV then computes the softmax denominator for free).
  S^T:  [k, q] tiles = K @ Q^T; the two heads of a chunk pair use array rows
        0-63 / 64-127 and run concurrently (PE row tiling).
  PV:   O^T [d+1, q] accumulated over k tiles; row 64 = rowsum; normalize
        with reciprocal_approx_fast + partition_broadcast + multiply.
  proj: out[t, c_out] partial accumulated over the 512 local c_in rows.
"""

import os
import sys

import numpy as np

for _p in ("/opt/trn_rl_repo", "/root/.axon_site/_ro/trn_rl_repo"):
    if os.path.isdir(_p) and _p not in sys.path:
        sys.path.append(_p)

import ml_dtypes
import concourse.bacc as bacc
import concourse.mybir as mybir
from concourse.tile import TileContext
from concourse.bass_utils import run_bass_kernel_spmd

B, T, C, H, D = 4, 2048, 1024, 16, 64
HL = 8            # heads per core
CL = HL * D       # 512: local qkv width
NCT = C // 128    # 8 contraction tiles over C
NQC = T // 512    # 4 query chunks
NKT = T // 128    # 16 key tiles
NCORES = 8

FP32 = mybir.dt.float32
BF16 = mybir.dt.bfloat16

LAST_EXEC_NS = None
_CACHE = {}


def _act_reciprocal(nc, out_ap, in_ap):
    """Raw ACT Reciprocal (bass's activation() refuses it; ~1e-5 rel err is
    plenty for softmax denominators and it is 5x cheaper than DVE reciprocal)."""
    inst = mybir.InstActivation(
        name=nc.get_next_instruction_name(),
        func=mybir.ActivationFunctionType.Reciprocal,
        ins=[nc.scalar.lower_ap(in_ap),
             mybir.ImmediateValue(dtype=mybir.dt.float32, value=0.0),
             mybir.ImmediateValue(dtype=mybir.dt.float32, value=1.0),
             mybir.ImmediateValue(dtype=mybir.dt.float32, value=0.0)],
        outs=[nc.scalar.lower_ap(out_ap)])
    return nc.scalar.add_instruction(inst)


def _build():
    nc = bacc.Bacc("TRN2", target_bir_lowering=False, debug=False)
    xT = nc.dram_tensor("xT", [C, T], BF16, kind="ExternalInput")
    wq = nc.dram_tensor("wq", [C, CL], BF16, kind="ExternalInput")
    wk = nc.dram_tensor("wk", [C, CL], BF16, kind="ExternalInput")
    wv = nc.dram_tensor("wv", [C, CL], BF16, kind="ExternalInput")
    wp = nc.dram_tensor("wp", [CL, C], BF16, kind="ExternalInput")
    bqk = nc.dram_tensor("bqk", [128, 8], FP32, kind="ExternalInput")
    bv = nc.dram_tensor("bv", [1, CL], FP32, kind="ExternalInput")
    maskb = nc.dram_tensor("maskb", [128, 4, 2, 512], BF16, kind="ExternalInput")
    out = nc.dram_tensor("out", [T, C], FP32, kind="ExternalOutput")

    EXP = mybir.ActivationFunctionType.Exp

    with TileContext(nc) as tc:
        with (
            tc.tile_pool(name="persist", bufs=1) as pp,
            tc.tile_pool(name="xtp", bufs=1) as xt_pool,
            tc.tile_pool(name="wc", bufs=4) as wc_pool,
            tc.tile_pool(name="wvp", bufs=1) as wv_pool,
            tc.tile_pool(name="wpp", bufs=1) as wp_pool,
            tc.tile_pool(name="yt", bufs=1) as yt_pool,
            tc.tile_pool(name="pt", bufs=8) as pt_pool,
            tc.tile_pool(name="nrm", bufs=6) as nrm_pool,
            tc.tile_pool(name="stg", bufs=3) as stg_pool,
            tc.tile_pool(name="psS", bufs=2, space="PSUM") as psS,   # attention S^T pairs
            tc.tile_pool(name="psF", bufs=2, space="PSUM") as psF,   # qkv/v/proj groups
            tc.tile_pool(name="psO", bufs=2, space="PSUM") as psO,   # PV accumulators
        ):
            # persistent SBUF; qkT chunks split per 1024-wide half for finer deps
            qkTn = [[pp.tile([128, 1024], BF16, name=f"qkT{m}_{np_}")
                     for np_ in range(2)] for m in range(8)]
            vt = [pp.tile([128, HL, D + 1], BF16, name=f"v{i}") for i in range(NKT)]
            bqk_sb = pp.tile([128, 8], FP32, name="bqk_sb")
            bv1_sb = pp.tile([1, CL], FP32, name="bv1_sb")
            bvb_sb = pp.tile([128, CL], FP32, name="bvb_sb")
            mask_sb = pp.tile([128, 4, 2, 512], BF16, name="mask_sb")

            nc.gpsimd.dma_start(bqk_sb[:], bqk[:])
            nc.gpsimd.dma_start(bv1_sb[:], bv[:])
            nc.gpsimd.dma_start(mask_sb[:], maskb[:])
            nc.gpsimd.partition_broadcast(bvb_sb[:], bv1_sb[:])
            for i in range(NKT):
                nc.vector.memset(vt[i][:, :, D:D + 1], 1.0)

            def wt_load(wdram, m):
                wt = wc_pool.tile([128, NCT, 128], BF16, name="wt", tag="wt")
                nc.gpsimd.dma_start(
                    wt[:],
                    wdram[:, m * 128:(m + 1) * 128].rearrange("(a p) n -> p a n", p=128),
                )
                return wt

            wt_q0 = wt_load(wq, 0)
            wt_k0 = wt_load(wk, 0)

            xt = [xt_pool.tile([128, T], BF16, name=f"xt{ct}") for ct in range(NCT)]
            for ct in range(NCT):
                nc.sync.dma_start(xt[ct][:], xT[ct * 128:(ct + 1) * 128, :])
            wvt = wv_pool.tile([128, NCT, CL], BF16, name="wvt")
            nc.gpsimd.dma_start(wvt[:], wv.rearrange("(a p) n -> p a n", p=128))

            # absorber: PE observes xt0's DMA queue before the first real
            # matmul (limited sync-wait slots on the weight-load path)
            dummy = psO.tile([1, 64], FP32, name="po", tag="po")
            nc.tensor.matmul(dummy[:], xt[0][:, 0:1], xt[0][:, 0:64])

            def qk_half(m, col, wt, np_):
                """One 1024-wide half of a q/k chunk: two 8-matmul groups."""
                for half in range(2):
                    n = 2 * np_ + half
                    ps = psF.tile([128, 512], FP32, name="fg", tag="fg")
                    for ct in range(NCT):
                        nc.tensor.matmul(
                            ps[:],
                            wt[:, ct, :],
                            xt[ct][:, n * 512:(n + 1) * 512],
                            start=(ct == 0), stop=(ct == NCT - 1),
                        )
                    nc.vector.tensor_scalar_add(
                        qkTn[col][np_][:, half * 512:(half + 1) * 512], ps[:],
                        bqk_sb[:, col:col + 1])

            def qk_half_gen(m, col, wt, np_):
                """Generator version: yields every 2 matmuls (feeder unit)."""
                for half in range(2):
                    n = 2 * np_ + half
                    ps = psF.tile([128, 512], FP32, name="fg", tag="fg")
                    for cp in range(NCT // 2):
                        for ct in (2 * cp, 2 * cp + 1):
                            nc.tensor.matmul(
                                ps[:],
                                wt[:, ct, :],
                                xt[ct][:, n * 512:(n + 1) * 512],
                                start=(ct == 0), stop=(ct == NCT - 1),
                            )
                        yield
                    nc.vector.tensor_scalar_add(
                        qkTn[col][np_][:, half * 512:(half + 1) * 512], ps[:],
                        bqk_sb[:, col:col + 1])
                    yield

            def v_group(ip):
                for half in range(2):
                    i = 2 * ip + half
                    ps = psF.tile([128, 512], FP32, name="fg", tag="fg")
                    for ct in range(NCT):
                        nc.tensor.matmul(
                            ps[:],
                            xt[ct][:, i * 128:(i + 1) * 128],
                            wvt[:, ct, :],
                            start=(ct == 0), stop=(ct == NCT - 1),
                        )
                    nc.vector.tensor_add(
                        vt[i][:, :, 0:D],
                        ps[:].rearrange("p (h d) -> p h d", h=HL),
                        bvb_sb[:].rearrange("p (h d) -> p h d", h=HL),
                    )

            feeder = []

            def pump(n):
                while n > 0 and feeder:
                    try:
                        next(feeder[0])
                        n -= 1
                    except StopIteration:
                        feeder.pop(0)

            yT = [yt_pool.tile([128, 4, 512], BF16, name=f"yT{j}") for j in range(NQC)]

            def attention(m, fillers=None, order=None):
                kcol = 4 + m
                for idx, j in enumerate(order or range(NQC)):
                    for fn in (fillers or {}).get(idx, []):
                        fn()
                    po = [psO.tile([65, 512], FP32, name="po", tag="po")
                          for _ in range(2)]
                    npair = 2 * (j + 1)
                    for kp in range(npair):
                        kts = (2 * kp, 2 * kp + 1)
                        starts = [min(max(0, 128 * (kts[half] - 4 * j)), 512)
                                  for half in range(2)]
                        # per k-tile (half): one [128,1024] tile holding both
                        # heads side by side (separate banks) so the two K=64
                        # matmuls are adjacent and row-pair on the PE array
                        ps = [psS.tile([128, 1024], FP32, name="st", tag="st")
                              for _ in range(2)]
                        for half in range(2):
                            s = starts[half]
                            kt = kts[half]
                            for hh in range(2):
                                pb = hh * 64
                                nc.tensor.matmul(
                                    ps[half][:, hh * 512 + s:(hh + 1) * 512],
                                    qkTn[kcol][kt // 8][pb:pb + 64,
                                        (kt % 8) * 128:(kt % 8 + 1) * 128],
                                    qkTn[m][j // 2][pb:pb + 64,
                                        (j % 2) * 512 + s:(j % 2 + 1) * 512],
                                    start=True, stop=True,
                                    tile_position=(pb, 0),
                                )
                        pt = [pt_pool.tile([128, 1024], BF16, name="pt", tag="pt")
                              for _ in range(2)]
                        for half in range(2):
                            s = starts[half]
                            if s == 0:
                                nc.scalar.activation(pt[half][:], ps[half][:], EXP)
                            else:
                                nc.scalar.activation(
                                    pt[half][:].rearrange(
                                        "p (h n) -> p h n", h=2)[:, :, s:512],
                                    ps[half][:].rearrange(
                                        "p (h n) -> p h n", h=2)[:, :, s:512],
                                    EXP)
                            if kts[half] >= 4 * j:
                                jj = kts[half] - 4 * j
                                pv3 = pt[half][:].rearrange(
                                    "p (h n) -> p h n", h=2)[:, :, s:512]
                                nc.vector.tensor_mul(
                                    pv3, pv3, mask_sb[:, jj, :, s:512])
                        for hh in range(2):
                            h = 2 * m + hh
                            for half in range(2):
                                s = starts[half]
                                nc.tensor.matmul(
                                    po[hh][:, s:512],
                                    vt[kts[half]][:, h, :],
                                    pt[half][:, hh * 512 + s:(hh + 1) * 512],
                                    start=(kp == 0 and half == 0),
                                    stop=(kp == npair - 1 and half == 1),
                                )
                        pump(2)
                    for hh in range(2):
                        pb = hh * 64
                        # copy out of PSUM right away so the po slot frees
                        oc = nrm_pool.tile([64, 512], FP32, name="oc", tag="oc")
                        nc.vector.tensor_copy(oc[:], po[hh][0:64, :])
                        rs = nrm_pool.tile([1, 512], FP32, name="rs", tag="rs")
                        nc.vector.tensor_copy(rs[:], po[hh][64:65, :])
                        recip = nrm_pool.tile([1, 512], FP32, name="recip", tag="recip")
                        nc.vector.reciprocal_approx_fast(recip[:], rs[:])
                        rb = nrm_pool.tile([64, 512], FP32, name="rb", tag="rb")
                        nc.gpsimd.partition_broadcast(rb[:], recip[:])
                        nc.vector.tensor_mul(
                            yT[j][pb:pb + 64, m, :], oc[:], rb[:]
                        )

            # ---------------- proj ----------------
            wpt = wp_pool.tile([128, 4, C], BF16, name="wpt")

            def proj_block(j):
                for nn in range(2):
                    for ts in range(4):
                        ps = psF.tile([128, 512], FP32, name="fg", tag="fg")
                        for cl in range(4):
                            nc.tensor.matmul(
                                ps[:],
                                yT[j][:, cl, ts * 128:(ts + 1) * 128],
                                wpt[:, cl, nn * 512:(nn + 1) * 512],
                                start=(cl == 0), stop=(cl == 3),
                            )
                        stage = stg_pool.tile([128, 512], FP32, name="stage", tag="stage")
                        nc.vector.tensor_copy(stage[:], ps[:])
                        nc.sync.dma_start(
                            out[j * 512 + ts * 128:j * 512 + (ts + 1) * 128,
                                nn * 512:(nn + 1) * 512],
                            stage[:],
                        )

            def qk_half_ctmajor(m, col, wt, np_):
                """ct-outer over both 512-halves: PE can start as soon as
                xt[0] lands instead of waiting for the whole x stream."""
                pss = [psF.tile([128, 512], FP32, name="fg", tag="fg")
                       for _ in range(2)]
                for ct in range(NCT):
                    for half in range(2):
                        n = 2 * np_ + half
                        nc.tensor.matmul(
                            pss[half][:],
                            wt[:, ct, :],
                            xt[ct][:, n * 512:(n + 1) * 512],
                            start=(ct == 0), stop=(ct == NCT - 1),
                        )
                for half in range(2):
                    nc.vector.tensor_scalar_add(
                        qkTn[col][np_][:, half * 512:(half + 1) * 512],
                        pss[half][:], bqk_sb[:, col:col + 1])

            # ---------- schedule ----------
            qk_half_ctmajor(0, 0, wt_q0, 0)      # q0 lower half
            qk_half_ctmajor(0, 4, wt_k0, 0)      # k0 lower half
            v_group(0)
            v_group(1)
            wt_q1 = wt_load(wq, 1)
            wt_k1 = wt_load(wk, 1)
            feeder.append(qk_half_gen(1, 1, wt_q1, 0))
            feeder.append(qk_half_gen(1, 5, wt_k1, 0))
            feeder.append(qk_half_gen(1, 1, wt_q1, 1))
            feeder.append(qk_half_gen(1, 5, wt_k1, 1))
            attention(0, {
                1: [lambda: v_group(2), lambda: v_group(3),
                    lambda: qk_half(0, 0, wt_q0, 1)],
                2: [lambda: v_group(4), lambda: v_group(5),
                    lambda: qk_half(0, 4, wt_k0, 1)],
                3: [lambda: v_group(6), lambda: v_group(7)],
            })
            wt_q2 = wt_load(wq, 2)
            wt_k2 = wt_load(wk, 2)
            feeder.append(qk_half_gen(2, 2, wt_q2, 0))
            feeder.append(qk_half_gen(2, 6, wt_k2, 0))
            feeder.append(qk_half_gen(2, 2, wt_q2, 1))
            feeder.append(qk_half_gen(2, 6, wt_k2, 1))
            attention(1)
            wt_q3 = wt_load(wq, 3)
            wt_k3 = wt_load(wk, 3)
            nc.gpsimd.dma_start(wpt[:], wp.rearrange("(a p) n -> p a n", p=128))
            feeder.append(qk_half_gen(3, 3, wt_q3, 0))
            feeder.append(qk_half_gen(3, 7, wt_k3, 0))
            feeder.append(qk_half_gen(3, 3, wt_q3, 1))
            feeder.append(qk_half_gen(3, 7, wt_k3, 1))
            attention(2)
            attention(3, {
                1: [lambda: proj_block(0)],
                2: [lambda: proj_block(1)],
                3: [lambda: proj_block(2)],
            })
            proj_block(3)
    nc.compile()
    return nc


def _host_inputs(x, W_attn, b_attn, W_proj):
    """Build the 8 per-core input maps (bf16 casts happen here)."""
    x = np.asarray(x, dtype=np.float32)
    W_attn = np.asarray(W_attn, dtype=np.float32)
    b_attn = np.asarray(b_attn, dtype=np.float32)
    W_proj = np.asarray(W_proj, dtype=np.float32)

    scale = np.float32(1.0 / np.sqrt(D))
    # causal mask for diagonal tiles: [p, jj, f] = 1 where f >= p + 128*jj
    p = np.arange(128)[:, None, None]
    jj = np.arange(4)[None, :, None]
    f = np.arange(512)[None, None, :]
    maskb = (f >= p + 128 * jj).astype(ml_dtypes.bfloat16)
    maskb = np.ascontiguousarray(np.repeat(maskb[:, :, None, :], 2, axis=2))

    bf = ml_dtypes.bfloat16
    in_maps = []
    xT_b = [np.ascontiguousarray(x[b].T.astype(bf)) for b in range(B)]
    for c in range(NCORES):
        b, hg = c // 2, c % 2
        lo = hg * CL
        wq_np = np.ascontiguousarray((W_attn[:, lo:lo + CL] * scale).astype(bf))
        wk_np = np.ascontiguousarray(W_attn[:, C + lo:C + lo + CL].astype(bf))
        wv_np = np.ascontiguousarray(W_attn[:, 2 * C + lo:2 * C + lo + CL].astype(bf))
        wp_np = np.ascontiguousarray(W_proj[lo:lo + CL, :].astype(bf))
        bq = (b_attn[lo:lo + CL] * scale).reshape(4, 128).T          # [128, 4]
        bk = b_attn[C + lo:C + lo + CL].reshape(4, 128).T
        bqk_np = np.ascontiguousarray(np.concatenate([bq, bk], axis=1), dtype=np.float32)
        bv_np = np.ascontiguousarray(b_attn[2 * C + lo:2 * C + lo + CL].reshape(1, CL))
        in_maps.append({
            "xT": xT_b[b],
            "wq": wq_np, "wk": wk_np, "wv": wv_np, "wp": wp_np,
            "bqk": bqk_np, "bv": bv_np, "maskb": maskb,
        })
    return in_maps


def kernel(x, W_attn, b_attn, W_proj, b_proj):
    global LAST_EXEC_NS
    if "nc" not in _CACHE:
        _CACHE["nc"] = _build()
    nc = _CACHE["nc"]
    in_maps = _host_inputs(x, W_attn, b_attn, W_proj)
    trace = os.environ.get("KERNEL_TRACE", "0") == "1"
    kwargs = {}
    if trace:
        kwargs["trace"] = True
        td = os.environ.get("KERNEL_TRACE_DIR")
        if td:
            kwargs["tmpdir"] = td
    res = run_bass_kernel_spmd(nc, in_maps, list(range(NCORES)), **kwargs)
    LAST_EXEC_NS = res.exec_time_ns
    b_proj = np.asarray(b_proj, dtype=np.float32)
    outs = []
    for b in range(B):
        outs.append(res.results[2 * b]["out"] + res.results[2 * b + 1]["out"] + b_proj)
    return np.stack(outs, axis=0).astype(np.float32)


# revision 26
# speedup vs baseline: 1.0470x; 1.0182x over previous
"""Causal self-attention (B=4, T=2048, C=1024, H=16) on 8 TRN2 NeuronCores.

Sharding: 2-D data x tensor parallel. Core c handles batch b = c//2 and
head-group hg = c%2 (8 of 16 heads). Each core computes its local QKV
projection (c_attn columns for its heads), causal attention for its 8 heads,
and a partial c_proj (rows for its heads). The pair of cores sharing a batch
produce partial outputs that are summed on the host (the gather step).

All matmul inputs are bf16 (cast on host / on-chip), accumulation is fp32 in
PSUM. 1/sqrt(D) is folded into W_q on the host. Softmax skips the max
subtraction: with this input distribution |scores| <= ||q||*||k||/8, far from
fp32 exp overflow. Causal masking is a post-exp multiply with a 0/1 bf16
triangular mask on diagonal tiles.

Per-core kernel (matmuls contract over the SBUF partition dim):
  QKV:  qT/kT chunks [d_local, t] = W.T @ x.T; v tiles [t, d] with a ones
        column per head (PV then computes the softmax denominator for free).
  S^T:  [k, q] tiles = K @ Q^T; the two heads of a chunk pair use array rows
        0-63 / 64-127 and run concurrently (PE row tiling).
  PV:   O^T [d+1, q] accumulated over k tiles; row 64 = rowsum; normalize
        with reciprocal_approx_fast + partition_broadcast + multiply.
  proj: out[t, c_out] partial accumulated over the 512 local c_in rows.
"""

import os
import sys

import numpy as np

for _p in ("/opt/trn_rl_repo", "/root/.axon_site/_ro/trn_rl_repo"):
    if os.path.isdir(_p) and _p not in sys.path:
        sys.path.append(_p)

import ml_dtypes
import concourse.bacc as bacc
import concourse.mybir as mybir
from concourse.tile import TileContext
from concourse.bass_utils import run_bass_kernel_spmd

B, T, C, H, D = 4, 2048, 1024, 16, 64
HL = 8            # heads per core
CL = HL * D       # 512: local qkv width
NCT = C // 128    # 8 contraction tiles over C
NQC = T // 512    # 4 query chunks
NKT = T // 128    # 16 key tiles
NCORES = 8

FP32 = mybir.dt.float32
BF16 = mybir.dt.bfloat16

LAST_EXEC_NS = None
_CACHE = {}


def _act_reciprocal(nc, out_ap, in_ap):
    """Raw ACT Reciprocal (bass's activation() refuses it; ~1e-5 rel err is
    plenty for softmax denominators and it is 5x cheaper than DVE reciprocal)."""
    inst = mybir.InstActivation(
        name=nc.get_next_instruction_name(),
        func=mybir.ActivationFunctionType.Reciprocal,
        ins=[nc.scalar.lower_ap(in_ap),
             mybir.ImmediateValue(dtype=mybir.dt.float32, value=0.0),
             mybir.ImmediateValue(dtype=mybir.dt.float32, value=1.0),
             mybir.ImmediateValue(dtype=mybir.dt.float32, value=0.0)],
        outs=[nc.scalar.lower_ap(out_ap)])
    return nc.scalar.add_instruction(inst)


def _build():
    nc = bacc.Bacc("TRN2", target_bir_lowering=False, debug=False)
    xT = nc.dram_tensor("xT", [C, T], BF16, kind="ExternalInput")
    wq = nc.dram_tensor("wq", [C, CL], BF16, kind="ExternalInput")
    wk = nc.dram_tensor("wk", [C, CL], BF16, kind="ExternalInput")
    wv = nc.dram_tensor("wv", [C, CL], BF16, kind="ExternalInput")
    wp = nc.dram_tensor("wp", [CL, C], BF16, kind="ExternalInput")
    bqk = nc.dram_tensor("bqk", [128, 8], FP32, kind="ExternalInput")
    bv = nc.dram_tensor("bv", [1, CL], FP32, kind="ExternalInput")
    maskb = nc.dram_tensor("maskb", [128, 4, 2, 512], BF16, kind="ExternalInput")
    out = nc.dram_tensor("out", [T, C], FP32, kind="ExternalOutput")

    EXP = mybir.ActivationFunctionType.Exp

    with TileContext(nc) as tc:
        with (
            tc.tile_pool(name="persist", bufs=1) as pp,
            tc.tile_pool(name="xtp", bufs=1) as xt_pool,
            tc.tile_pool(name="wc", bufs=4) as wc_pool,
            tc.tile_pool(name="wvp", bufs=1) as wv_pool,
            tc.tile_pool(name="wpp", bufs=1) as wp_pool,
            tc.tile_pool(name="yt", bufs=1) as yt_pool,
            tc.tile_pool(name="pt", bufs=8) as pt_pool,
            tc.tile_pool(name="nrm", bufs=6) as nrm_pool,
            tc.tile_pool(name="stg", bufs=3) as stg_pool,
            tc.tile_pool(name="psS", bufs=2, space="PSUM") as psS,   # attention S^T pairs
            tc.tile_pool(name="psF", bufs=2, space="PSUM") as psF,   # qkv/v/proj groups
            tc.tile_pool(name="psO", bufs=2, space="PSUM") as psO,   # PV accumulators
        ):
            # persistent SBUF; qkT chunks split per 1024-wide half for finer deps
            qkTn = [[pp.tile([128, 1024], BF16, name=f"qkT{m}_{np_}")
                     for np_ in range(2)] for m in range(8)]
            vt = [pp.tile([128, HL, D + 1], BF16, name=f"v{i}") for i in range(NKT)]
            bqk_sb = pp.tile([128, 8], FP32, name="bqk_sb")
            bv1_sb = pp.tile([1, CL], FP32, name="bv1_sb")
            bvb_sb = pp.tile([128, CL], FP32, name="bvb_sb")
            mask_sb = pp.tile([128, 4, 2, 512], BF16, name="mask_sb")

            nc.gpsimd.dma_start(bqk_sb[:], bqk[:])
            nc.gpsimd.dma_start(bv1_sb[:], bv[:])
            nc.gpsimd.dma_start(mask_sb[:], maskb[:])
            nc.gpsimd.partition_broadcast(bvb_sb[:], bv1_sb[:])
            for i in range(NKT):
                nc.vector.memset(vt[i][:, :, D:D + 1], 1.0)

            def wt_load(wdram, m):
                wt = wc_pool.tile([128, NCT, 128], BF16, name="wt", tag="wt")
                nc.sync.dma_start(
                    wt[:],
                    wdram[:, m * 128:(m + 1) * 128].rearrange("(a p) n -> p a n", p=128),
                )
                return wt

            wt_q0 = wt_load(wq, 0)
            wt_k0 = wt_load(wk, 0)

            xt = [xt_pool.tile([128, T], BF16, name=f"xt{ct}") for ct in range(NCT)]
            for ct in range(NCT):
                nc.sync.dma_start(xt[ct][:], xT[ct * 128:(ct + 1) * 128, :])
            wvt = wv_pool.tile([128, NCT, CL], BF16, name="wvt")
            nc.sync.dma_start(wvt[:], wv.rearrange("(a p) n -> p a n", p=128))

            # absorber: PE observes xt0's DMA queue before the first real
            # matmul (limited sync-wait slots on the weight-load path)
            dummy = psO.tile([1, 64], FP32, name="po", tag="po")
            nc.tensor.matmul(dummy[:], xt[0][:, 0:1], xt[0][:, 0:64])

            def qk_half(m, col, wt, np_):
                """One 1024-wide half of a q/k chunk: two 8-matmul groups."""
                for half in range(2):
                    n = 2 * np_ + half
                    ps = psF.tile([128, 512], FP32, name="fg", tag="fg")
                    for ct in range(NCT):
                        nc.tensor.matmul(
                            ps[:],
                            wt[:, ct, :],
                            xt[ct][:, n * 512:(n + 1) * 512],
                            start=(ct == 0), stop=(ct == NCT - 1),
                        )
                    nc.vector.tensor_scalar_add(
                        qkTn[col][np_][:, half * 512:(half + 1) * 512], ps[:],
                        bqk_sb[:, col:col + 1])

            def qk_half_gen(m, col, wt, np_):
                """Generator version: yields every 2 matmuls (feeder unit)."""
                for half in range(2):
                    n = 2 * np_ + half
                    ps = psF.tile([128, 512], FP32, name="fg", tag="fg")
                    for cp in range(NCT // 2):
                        for ct in (2 * cp, 2 * cp + 1):
                            nc.tensor.matmul(
                                ps[:],
                                wt[:, ct, :],
                                xt[ct][:, n * 512:(n + 1) * 512],
                                start=(ct == 0), stop=(ct == NCT - 1),
                            )
                        yield
                    nc.vector.tensor_scalar_add(
                        qkTn[col][np_][:, half * 512:(half + 1) * 512], ps[:],
                        bqk_sb[:, col:col + 1])
                    yield

            def v_group(ip):
                for half in range(2):
                    i = 2 * ip + half
                    ps = psF.tile([128, 512], FP32, name="fg", tag="fg")
                    for ct in range(NCT):
                        nc.tensor.matmul(
                            ps[:],
                            xt[ct][:, i * 128:(i + 1) * 128],
                            wvt[:, ct, :],
                            start=(ct == 0), stop=(ct == NCT - 1),
                        )
                    nc.vector.tensor_add(
                        vt[i][:, :, 0:D],
                        ps[:].rearrange("p (h d) -> p h d", h=HL),
                        bvb_sb[:].rearrange("p (h d) -> p h d", h=HL),
                    )

            feeder = []

            def pump(n):
                while n > 0 and feeder:
                    try:
                        next(feeder[0])
                        n -= 1
                    except StopIteration:
                        feeder.pop(0)

            yT = [yt_pool.tile([128, 4, 512], BF16, name=f"yT{j}") for j in range(NQC)]

            def attention(m, fillers=None, order=None):
                kcol = 4 + m
                for idx, j in enumerate(order or range(NQC)):
                    for fn in (fillers or {}).get(idx, []):
                        fn()
                    po = [psO.tile([65, 512], FP32, name="po", tag="po")
                          for _ in range(2)]
                    npair = 2 * (j + 1)
                    for kp in range(npair):
                        kts = (2 * kp, 2 * kp + 1)
                        starts = [min(max(0, 128 * (kts[half] - 4 * j)), 512)
                                  for half in range(2)]
                        # per k-tile (half): one [128,1024] tile holding both
                        # heads side by side (separate banks) so the two K=64
                        # matmuls are adjacent and row-pair on the PE array
                        ps = [psS.tile([128, 1024], FP32, name="st", tag="st")
                              for _ in range(2)]
                        for half in range(2):
                            s = starts[half]
                            kt = kts[half]
                            for hh in range(2):
                                pb = hh * 64
                                nc.tensor.matmul(
                                    ps[half][:, hh * 512 + s:(hh + 1) * 512],
                                    qkTn[kcol][kt // 8][pb:pb + 64,
                                        (kt % 8) * 128:(kt % 8 + 1) * 128],
                                    qkTn[m][j // 2][pb:pb + 64,
                                        (j % 2) * 512 + s:(j % 2 + 1) * 512],
                                    start=True, stop=True,
                                    tile_position=(pb, 0),
                                )
                        pt = [pt_pool.tile([128, 1024], BF16, name="pt", tag="pt")
                              for _ in range(2)]
                        for half in range(2):
                            s = starts[half]
                            if s == 0:
                                nc.scalar.activation(pt[half][:], ps[half][:], EXP)
                            else:
                                nc.scalar.activation(
                                    pt[half][:].rearrange(
                                        "p (h n) -> p h n", h=2)[:, :, s:512],
                                    ps[half][:].rearrange(
                                        "p (h n) -> p h n", h=2)[:, :, s:512],
                                    EXP)
                            if kts[half] >= 4 * j:
                                jj = kts[half] - 4 * j
                                pv3 = pt[half][:].rearrange(
                                    "p (h n) -> p h n", h=2)[:, :, s:512]
                                nc.vector.tensor_mul(
                                    pv3, pv3, mask_sb[:, jj, :, s:512])
                        for hh in range(2):
                            h = 2 * m + hh
                            for half in range(2):
                                s = starts[half]
                                nc.tensor.matmul(
                                    po[hh][:, s:512],
                                    vt[kts[half]][:, h, :],
                                    pt[half][:, hh * 512 + s:(hh + 1) * 512],
                                    start=(kp == 0 and half == 0),
                                    stop=(kp == npair - 1 and half == 1),
                                )
                        pump(2)
                    for hh in range(2):
                        pb = hh * 64
                        # copy out of PSUM right away so the po slot frees
                        oc = nrm_pool.tile([64, 512], FP32, name="oc", tag="oc")
                        nc.vector.tensor_copy(oc[:], po[hh][0:64, :])
                        rs = nrm_pool.tile([1, 512], FP32, name="rs", tag="rs")
                        nc.vector.tensor_copy(rs[:], po[hh][64:65, :])
                        recip = nrm_pool.tile([1, 512], FP32, name="recip", tag="recip")
                        nc.vector.reciprocal_approx_fast(recip[:], rs[:])
                        rb = nrm_pool.tile([64, 512], FP32, name="rb", tag="rb")
                        nc.gpsimd.partition_broadcast(rb[:], recip[:])
                        nc.vector.tensor_mul(
                            yT[j][pb:pb + 64, m, :], oc[:], rb[:]
                        )

            # ---------------- proj ----------------
            wpt = wp_pool.tile([128, 4, C], BF16, name="wpt")

            def proj_block(j):
                for nn in range(2):
                    for ts in range(4):
                        ps = psF.tile([128, 512], FP32, name="fg", tag="fg")
                        for cl in range(4):
                            nc.tensor.matmul(
                                ps[:],
                                yT[j][:, cl, ts * 128:(ts + 1) * 128],
                                wpt[:, cl, nn * 512:(nn + 1) * 512],
                                start=(cl == 0), stop=(cl == 3),
                            )
                        stage = stg_pool.tile([128, 512], FP32, name="stage", tag="stage")
                        nc.vector.tensor_copy(stage[:], ps[:])
                        nc.sync.dma_start(
                            out[j * 512 + ts * 128:j * 512 + (ts + 1) * 128,
                                nn * 512:(nn + 1) * 512],
                            stage[:],
                        )

            def qk_half_ctmajor(m, col, wt, np_):
                """ct-outer over both 512-halves: PE can start as soon as
                xt[0] lands instead of waiting for the whole x stream."""
                pss = [psF.tile([128, 512], FP32, name="fg", tag="fg")
                       for _ in range(2)]
                for ct in range(NCT):
                    for half in range(2):
                        n = 2 * np_ + half
                        nc.tensor.matmul(
                            pss[half][:],
                            wt[:, ct, :],
                            xt[ct][:, n * 512:(n + 1) * 512],
                            start=(ct == 0), stop=(ct == NCT - 1),
                        )
                for half in range(2):
                    nc.vector.tensor_scalar_add(
                        qkTn[col][np_][:, half * 512:(half + 1) * 512],
                        pss[half][:], bqk_sb[:, col:col + 1])

            # ---------- schedule ----------
            qk_half_ctmajor(0, 0, wt_q0, 0)      # q0 lower half
            qk_half_ctmajor(0, 4, wt_k0, 0)      # k0 lower half
            v_group(0)
            v_group(1)
            wt_q1 = wt_load(wq, 1)
            wt_k1 = wt_load(wk, 1)
            feeder.append(qk_half_gen(1, 1, wt_q1, 0))
            feeder.append(qk_half_gen(1, 5, wt_k1, 0))
            feeder.append(qk_half_gen(1, 1, wt_q1, 1))
            feeder.append(qk_half_gen(1, 5, wt_k1, 1))
            attention(0, {
                1: [lambda: v_group(2), lambda: v_group(3),
                    lambda: qk_half(0, 0, wt_q0, 1)],
                2: [lambda: v_group(4), lambda: v_group(5),
                    lambda: qk_half(0, 4, wt_k0, 1)],
                3: [lambda: v_group(6), lambda: v_group(7)],
            })
            wt_q2 = wt_load(wq, 2)
            wt_k2 = wt_load(wk, 2)
            feeder.append(qk_half_gen(2, 2, wt_q2, 0))
            feeder.append(qk_half_gen(2, 6, wt_k2, 0))
            feeder.append(qk_half_gen(2, 2, wt_q2, 1))
            feeder.append(qk_half_gen(2, 6, wt_k2, 1))
            attention(1)
            wt_q3 = wt_load(wq, 3)
            wt_k3 = wt_load(wk, 3)
            nc.sync.dma_start(wpt[:], wp.rearrange("(a p) n -> p a n", p=128))
            feeder.append(qk_half_gen(3, 3, wt_q3, 0))
            feeder.append(qk_half_gen(3, 7, wt_k3, 0))
            feeder.append(qk_half_gen(3, 3, wt_q3, 1))
            feeder.append(qk_half_gen(3, 7, wt_k3, 1))
            attention(2)
            attention(3, {
                1: [lambda: proj_block(0)],
                2: [lambda: proj_block(1)],
                3: [lambda: proj_block(2)],
            })
            proj_block(3)
    nc.compile()
    return nc


def _host_inputs(x, W_attn, b_attn, W_proj):
    """Build the 8 per-core input maps (bf16 casts happen here)."""
    x = np.asarray(x, dtype=np.float32)
    W_attn = np.asarray(W_attn, dtype=np.float32)
    b_attn = np.asarray(b_attn, dtype=np.float32)
    W_proj = np.asarray(W_proj, dtype=np.float32)

    scale = np.float32(1.0 / np.sqrt(D))
    # causal mask for diagonal tiles: [p, jj, f] = 1 where f >= p + 128*jj
    p = np.arange(128)[:, None, None]
    jj = np.arange(4)[None, :, None]
    f = np.arange(512)[None, None, :]
    maskb = (f >= p + 128 * jj).astype(ml_dtypes.bfloat16)
    maskb = np.ascontiguousarray(np.repeat(maskb[:, :, None, :], 2, axis=2))

    bf = ml_dtypes.bfloat16
    in_maps = []
    xT_b = [np.ascontiguousarray(x[b].T.astype(bf)) for b in range(B)]
    for c in range(NCORES):
        b, hg = c // 2, c % 2
        lo = hg * CL
        wq_np = np.ascontiguousarray((W_attn[:, lo:lo + CL] * scale).astype(bf))
        wk_np = np.ascontiguousarray(W_attn[:, C + lo:C + lo + CL].astype(bf))
        wv_np = np.ascontiguousarray(W_attn[:, 2 * C + lo:2 * C + lo + CL].astype(bf))
        wp_np = np.ascontiguousarray(W_proj[lo:lo + CL, :].astype(bf))
        bq = (b_attn[lo:lo + CL] * scale).reshape(4, 128).T          # [128, 4]
        bk = b_attn[C + lo:C + lo + CL].reshape(4, 128).T
        bqk_np = np.ascontiguousarray(np.concatenate([bq, bk], axis=1), dtype=np.float32)
        bv_np = np.ascontiguousarray(b_attn[2 * C + lo:2 * C + lo + CL].reshape(1, CL))
        in_maps.append({
            "xT": xT_b[b],
            "wq": wq_np, "wk": wk_np, "wv": wv_np, "wp": wp_np,
            "bqk": bqk_np, "bv": bv_np, "maskb": maskb,
        })
    return in_maps


def kernel(x, W_attn, b_attn, W_proj, b_proj):
    global LAST_EXEC_NS
    if "nc" not in _CACHE:
        _CACHE["nc"] = _build()
    nc = _CACHE["nc"]
    in_maps = _host_inputs(x, W_attn, b_attn, W_proj)
    trace = os.environ.get("KERNEL_TRACE", "0") == "1"
    kwargs = {}
    if trace:
        kwargs["trace"] = True
        td = os.environ.get("KERNEL_TRACE_DIR")
        if td:
            kwargs["tmpdir"] = td
    res = run_bass_kernel_spmd(nc, in_maps, list(range(NCORES)), **kwargs)
    LAST_EXEC_NS = res.exec_time_ns
    b_proj = np.asarray(b_proj, dtype=np.float32)
    outs = []
    for b in range(B):
        outs.append(res.results[2 * b]["out"] + res.results[2 * b + 1]["out"] + b_proj)
    return np.stack(outs, axis=0).astype(np.float32)
